# revision 51
# baseline (speedup 1.0000x reference)
"""EnsembleMLP fused kernel for Trainium2, 8 NeuronCores (SPMD, batch-parallel).

Math transformation
-------------------
reference:
    hidden = tanh(x @ W_in.T)                                   [B, H]
    feats[b,m,e] = hidden[b, ids[m,e]]                          [B, M, E]
    preds[b,m,o] = sum_e feats[b,m,e] * W_pred[m,o,e]           [B, M, O]
    out = preds.mean(axis=1)                                    [B, O]

The gather + per-member matmul + mean are all linear in `hidden`, so they
collapse into a single [H, O] matrix:
    A[h,o] = (1/M) * sum_{(m,e): ids[m,e]==h} W_pred[m,o,e]
    out    = tanh(x @ W_in.T) @ A

A is built on the host from the tiny W_pred/ids tensors; the device does the
two matmuls + tanh. Sharding: data-parallel over batch — each of the 8 cores
takes 512 rows of x; W_in^T and A are replicated. No collectives.

Device schedule (per core)
--------------------------
All inputs are host-packed into ONE partition-major DRAM tensor `stream`
[128, 18752] bf16 whose column order equals consumption order. Measured HW
facts this schedule is built around:
  * The three dynamic DMA queues (Sync/Scalar/GpSimd rings) share one
    physical SDMA row set: ~330-370 GB/s aggregate; the Sync ring starts
    draining ~1 us after its doorbell while the Scalar/GpSimd rings take
    ~2.5-4 us to activate.
  * A lone N=512 matmul costs (398+N)/2.4 ns; a pipelined stream costs
    N/2.4+3 ns per matmul. So the second layer must run as a contiguous
    pipelined phase, not interleaved between L1 tiles.
  * The PE HAM clock gate needs ~3.4 us of sustained nonzero-data matmul
    activity before the PE runs at 2.4 GHz instead of 1.2 GHz.

  DMA: F0=[xt,g0] F1=[g1,aw] F2=[g2,g3] F5=[g16..g31] on the Sync ring
  (critical path + bulk tail), F3=[g4..g9] on Scalar, F4=[g10..g15] on
  GpSimd — the two slow-activating rings carry mid-stream tiles that are
  not needed until ~12 us in.

  L1: H^T tile t [h=128, b=512] = sum_n (wt chunk t,n).T @ (xt chunk n),
      psum bank t%4, tanh on ACT -> ht_sb[t] (bf16). 128 matmuls at the
      streaming rate (~218 ns each).
  L2: 4-way PE column tiling over h-tiles, N=512: quad q runs tiles
      4q..4q+3 concurrently in column groups 0..3 (tile_position (0,32j)),
      each group accumulating into its own PSUM bank at partitions
      [32j, 32j+10). 8 quads x ~216 ns.
  Tail: DVE add tree (pso0+pso1, pso2+pso3, sum) -> out_sb [10,512] f32,
      single DMA out. Host transposes back.
"""

import os

import numpy as np
import ml_dtypes

BATCH, IN_DIM, HIDDEN, N_MEMBERS, ENS, OUT = 4096, 512, 4096, 256, 64, 10
NCORES = 8
B_LOC = BATCH // NCORES      # 512 batch rows per core
HT = 128                     # h-tile height (PSUM partition dim)
NHT = HIDDEN // HT           # 32 h-tiles
NIC = IN_DIM // 128          # 4 contraction chunks for the first matmul
N_WARM = 38                  # warm-up matmuls (N=128) to lift the HAM gate

# ---- stream column layout (bf16 elems per partition), consumption order --
# [q0 q1 g0] [q2 q3] [g1 aw] [g2 g3] [g4..g9] [g10..g15] [g16..g23] [g24..g31]
XT_OFF = [0, 512, 1536, 2048]
AW_OFF = 3072
WT_OFF = {0: 1024, 1: 2560, 2: 3392, 3: 3904}
for _t in range(4, NHT):
    WT_OFF[_t] = 4416 + (_t - 4) * 512
STREAM_COLS = 18752
# (col_start, col_end, ring): A=Sync, B=Scalar, C=GpSimd.
# The Sync ring activates ~1us after its doorbell but gets starved by the
# SDMA arbiter when the other rings have work, so it carries the
# latency-critical head+mid while Scalar/GpSimd (2.5-4us activation lag,
# deliberately late doorbells) carry the two tail blocks.
DMA_SPLITS = [
    (0, 1536, "A"),       # F0a: q0 q1 g0   (critical head, first matmuls)
    (1536, 2560, "A"),    # F0b: q2 q3
    (2560, 3392, "A"),    # F1: g1 aw
    (3392, 4416, "A"),    # F2: g2 g3
    (4416, 7488, "A"),    # F3: g4..g9
    (7488, 10560, "A"),   # F4: g10..g15
    (10560, 14656, "B"),  # F5: g16..g23
    (14656, 18752, "C"),  # F6: g24..g31
]
# first L1 tile that needs DMA i (F0a/F0b are handled per-matmul in tile 0)
TILE_DMA = {1: 2, 2: 3, 4: 4, 10: 5, 16: 6, 24: 7}

_compiled = None
LAST_RESULT = None           # BassKernelResults of the most recent run


def _build():
    """Hand-scheduled raw Bass (no Tile framework — its prologue/epilogue
    cost ~12us on a kernel this size). 12 manual semaphores; the
    walrus-injected epilogue handles queue drain + semaphore-file reset."""
    from concourse import bacc, mybir

    bf16 = mybir.dt.bfloat16
    f32 = mybir.dt.float32

    nc = bacc.Bacc(
        "TRN2",
        target_bir_lowering=False,
        debug=False,
        enable_asserts=False,
        num_devices=NCORES,
    )
    stream = nc.dram_tensor("stream", [128, STREAM_COLS], bf16, kind="ExternalInput")
    out = nc.dram_tensor("out", [OUT, 4 * B_LOC], bf16, kind="ExternalOutput")

    warm_sb = nc.alloc_sbuf_tensor("warm_sb", [128, 128], mybir.dt.uint16)
    stream_sb = nc.alloc_sbuf_tensor("stream_sb", [128, STREAM_COLS], bf16)
    ht_sb = [
        nc.alloc_sbuf_tensor(f"ht_sb{t}", [128, B_LOC], bf16) for t in range(NHT)
    ]
    out_sb = nc.alloc_sbuf_tensor("out_sb", [OUT, 4 * B_LOC], bf16)

    # 8 PSUM banks: 4 for L1 accumulation (t%4), 4 for the L2 column groups
    # (group j accumulates h-tiles k≡j mod 4 at partitions [32j, 32j+10)).
    # Warm-up matmuls borrow ps[3] (first real use is L1 t=3).
    ps = [nc.alloc_psum_tensor(f"ps{k}", [128, B_LOC], f32) for k in range(4)]
    pso = [nc.alloc_psum_tensor(f"pso{j}", [128, B_LOC], f32) for j in range(4)]

    s_d = [nc.alloc_semaphore(f"s_d{i}") for i in range(len(DMA_SPLITS))]
    s_out = nc.alloc_semaphore("s_out")
    sg = nc.alloc_semaphore("sg")    # warm tile ready
    sm = nc.alloc_semaphore("sm")    # L1 tile t accumulated
    sa = nc.alloc_semaphore("sa")    # tanh t done
    sm2 = nc.alloc_semaphore("sm2")  # all L2 matmuls done
    sv = nc.alloc_semaphore("sv")    # L2 accumulator copies done

    tanh = mybir.ActivationFunctionType.Tanh

    # ---- input DMAs
    sview = stream.ap()

    def dma(i):
        c0, c1, ring = DMA_SPLITS[i]
        eng = {"A": nc.sync, "B": nc.scalar, "C": nc.gpsimd}[ring]
        eng.dma_start(
            out=stream_sb.ap()[:, c0:c1], in_=sview[:, c0:c1]
        ).then_inc(s_d[i], 16)

    for i in (0, 1, 2, 3, 4, 5):
        dma(i)                             # Sync ring, consumption order

    # ---- DVE: warm-up fill. The PE HAM clock-gate watches real datapath
    # activity (zeros don't count), so the warm-up needs varying nonzero
    # data: random bits masked to bf16 in [1, 2). Warm-up matmuls measured
    # NOT to slow the concurrent head DMA (v2 vs v4 traces: ~190 GB/s both).
    if os.environ.get("KERNEL_SIMSAFE") == "1":
        fill = nc.vector.memset(warm_sb.ap(), 0x3F80)  # CoreSim xorwow workaround
    else:
        fill = nc.vector.random(warm_sb.ap())
    fill.then_inc(sg, 1)
    nc.vector.wait_ge(sg, 1)
    nc.vector.tensor_scalar(
        out=warm_sb.ap(),
        in0=warm_sb.ap(),
        scalar1=0x007F,
        scalar2=0x3F80,
        op0=mybir.AluOpType.bitwise_and,
        op1=mybir.AluOpType.bitwise_or,
    ).then_inc(sg, 1)

    # ---- PE
    pe = nc.tensor
    pe.wait_ge(sg, 2)
    warm_bf = warm_sb.ap().bitcast(bf16)
    for _ in range(N_WARM):
        pe.matmul(
            out=ps[3].ap()[:, :128], lhsT=warm_bf, rhs=warm_bf,
            start=True, stop=True,
        )

    # L1: 128 back-to-back N=512 matmuls
    for t in range(NHT):
        if t in TILE_DMA:
            pe.wait_ge(s_d[TILE_DMA[t]], 16)
        if t >= 4:
            pe.wait_ge(sa, t - 3)          # psum bank free after tanh(t-4)
        for n in range(NIC):
            if t == 0 and n == 0:
                pe.wait_ge(s_d[0], 16)     # q0 q1 g0
            if t == 0 and n == 2:
                pe.wait_ge(s_d[1], 16)     # q2 q3
            mm = pe.matmul(
                out=ps[t % 4].ap(),
                lhsT=stream_sb.ap()[:, WT_OFF[t] + 128 * n : WT_OFF[t] + 128 * (n + 1)],
                rhs=stream_sb.ap()[:, XT_OFF[n] : XT_OFF[n] + B_LOC],
                start=(n == 0),
                stop=(n == NIC - 1),
            )
        mm.then_inc(sm, 1)

    # L2 phase: 8 pipelined quads, 4 column groups x N=512 each
    for q in range(NHT // 4):
        pe.wait_ge(sa, 4 * q + 4)          # tanh of tiles 4q..4q+3 done
        for j in range(4):
            k = 4 * q + j
            mm = pe.matmul(
                out=pso[j].ap()[32 * j : 32 * j + OUT, :],
                lhsT=stream_sb.ap()[:, AW_OFF + OUT * k : AW_OFF + OUT * (k + 1)],
                rhs=ht_sb[k].ap(),
                start=(q == 0),
                stop=(q == NHT // 4 - 1),
                tile_position=(0, 32 * j),
            )
    mm.then_inc(sm2, 1)                    # MMs retire in pc order

    # ---- ACT: issue the Scalar-ring input DMAs, then tanh PSUM->SBUF per
    # h-tile (act table preloaded by the compiler-injected ACT_TABLE_LOAD),
    # then evacuate L2 groups 0/2 while DVE evacuates 1/3 (different banks;
    # the four partial sums are added on the host — copies run at the DVE/
    # ACT fast-copy rate while adds would cost ~2.5x).
    act = nc.scalar
    act.wait_ge(sm, 1)
    dma(6)                                 # F5 doorbell after tile 0
    for t in range(NHT):
        if t > 0:
            act.wait_ge(sm, t + 1)
        act.activation(out=ht_sb[t].ap(), in_=ps[t % 4].ap(), func=tanh).then_inc(
            sa, 1
        )
    # ---- GpSimd: late tail-block doorbell (SWDGE), same contention logic
    nc.gpsimd.wait_ge(sm, 4)
    dma(7)

    # ---- DVE: evacuate all four L2 accumulators (DVE fast-copy ~280ns
    # beats ACT's ~700ns ACTIVATE; four on DVE finish sooner than 2+2)
    v = nc.vector
    v.wait_ge(sm2, 1)
    for j in range(4):
        cv = v.tensor_copy(
            out=out_sb.ap()[:, j * B_LOC : (j + 1) * B_LOC],
            in_=pso[j].ap()[32 * j : 32 * j + OUT, :],
        )
    cv.then_inc(sv, 1)

    # ---- Sync tail: result out. No completion wait / sem reset — the
    # injected per-engine epilogue drains every queue after program end.
    nc.sync.wait_ge(sv, 1)
    nc.sync.dma_start(out=out.ap(), in_=out_sb.ap()).then_inc(s_out, 16)

    nc.compile()
    return nc


def _pack_host(x, W_in, W_pred, ids):
    """Build the per-core stream tensors from a shared template."""
    A = np.zeros((HIDDEN, OUT), dtype=np.float64)
    np.add.at(
        A,
        ids.reshape(-1),
        W_pred.transpose(0, 2, 1).reshape(-1, OUT).astype(np.float64),
    )
    A /= N_MEMBERS
    a_packed = np.ascontiguousarray(
        A.reshape(NHT, 128, OUT).transpose(1, 0, 2).reshape(128, NHT * OUT)
    ).astype(ml_dtypes.bfloat16)

    wt_bf = W_in.T.astype(ml_dtypes.bfloat16)                  # [512, 4096]
    # wt tile t chunk: [p, n*128+h] = W_in[t*128+h, n*128+p]
    wt_tiles = wt_bf.reshape(NIC, 128, NHT, HT).transpose(1, 2, 0, 3)  # [p,t,n,h]

    template = np.zeros((128, STREAM_COLS), dtype=ml_dtypes.bfloat16)
    for t in range(NHT):
        template[:, WT_OFF[t] : WT_OFF[t] + 512] = wt_tiles[:, t].reshape(128, 512)
    template[:, AW_OFF : AW_OFF + NHT * OUT] = a_packed

    xt_bf = x.T.astype(ml_dtypes.bfloat16)                     # [512, 4096]
    streams = []
    for c in range(NCORES):
        xs = xt_bf[:, c * B_LOC : (c + 1) * B_LOC]             # [512, 512]
        xq = xs.reshape(NIC, 128, B_LOC)                       # [n, p, b]
        s = template.copy()
        for n in range(NIC):
            s[:, XT_OFF[n] : XT_OFF[n] + B_LOC] = xq[n]
        streams.append(s)
    return streams


def kernel(**inputs) -> np.ndarray:
    x = np.asarray(inputs["x"], dtype=np.float32)              # [4096, 512]
    W_in = np.asarray(inputs["W_in"], dtype=np.float32)        # [4096, 512]
    W_pred = np.asarray(inputs["W_pred"], dtype=np.float32)    # [256, 10, 64]
    ids = np.asarray(inputs["ensemble_input_ids"])             # [256, 64] int32

    streams = _pack_host(x, W_in, W_pred, ids)

    global _compiled
    if _compiled is None:
        _compiled = _build()
    nc = _compiled

    in_maps = [{"stream": streams[c]} for c in range(NCORES)]

    from concourse.bass_utils import run_bass_kernel_spmd

    trace = bool(int(os.environ.get("KERNEL_TRACE", "0")))
    res = run_bass_kernel_spmd(
        nc, in_maps, core_ids=list(range(NCORES)), trace=trace
    )
    global LAST_RESULT
    LAST_RESULT = res

    out = np.empty((BATCH, OUT), dtype=np.float32)
    for c in range(NCORES):
        arr = np.asarray(res.results[c]["out"], dtype=np.float32)  # [10, 2048]
        acc = (
            arr[:, 0:B_LOC]
            + arr[:, B_LOC : 2 * B_LOC]
            + arr[:, 2 * B_LOC : 3 * B_LOC]
            + arr[:, 3 * B_LOC :]
        )
        out[c * B_LOC : (c + 1) * B_LOC, :] = acc.T
    return out


# revision 57
# speedup vs baseline: 1.0836x; 1.0836x over previous
"""EnsembleMLP fused kernel for Trainium2, 8 NeuronCores (SPMD, batch-parallel).

Math transformation
-------------------
reference:
    hidden = tanh(x @ W_in.T)                                   [B, H]
    feats[b,m,e] = hidden[b, ids[m,e]]                          [B, M, E]
    preds[b,m,o] = sum_e feats[b,m,e] * W_pred[m,o,e]           [B, M, O]
    out = preds.mean(axis=1)                                    [B, O]

The gather + per-member matmul + mean are all linear in `hidden`, so they
collapse into a single [H, O] matrix:
    A[h,o] = (1/M) * sum_{(m,e): ids[m,e]==h} W_pred[m,o,e]
    out    = tanh(x @ W_in.T) @ A

A is built on the host from the tiny W_pred/ids tensors; the device does the
two matmuls + tanh. Sharding: data-parallel over batch — each of the 8 cores
takes 512 rows of x; W_in^T and A are replicated. No collectives.

Device schedule (per core)
--------------------------
All inputs are host-packed into ONE partition-major DRAM tensor `stream`
[128, 18752] bf16 whose column order equals consumption order. Measured HW
facts this schedule is built around:
  * The three dynamic DMA queues (Sync/Scalar/GpSimd rings) share one
    physical SDMA row set: ~330-370 GB/s aggregate; the Sync ring starts
    draining ~1 us after its doorbell while the Scalar/GpSimd rings take
    ~2.5-4 us to activate.
  * A lone N=512 matmul costs (398+N)/2.4 ns; a pipelined stream costs
    N/2.4+3 ns per matmul. So the second layer must run as a contiguous
    pipelined phase, not interleaved between L1 tiles.
  * The PE HAM clock gate needs ~3.4 us of sustained nonzero-data matmul
    activity before the PE runs at 2.4 GHz instead of 1.2 GHz.

  DMA: F0=[xt,g0] F1=[g1,aw] F2=[g2,g3] F5=[g16..g31] on the Sync ring
  (critical path + bulk tail), F3=[g4..g9] on Scalar, F4=[g10..g15] on
  GpSimd — the two slow-activating rings carry mid-stream tiles that are
  not needed until ~12 us in.

  L1: H^T tile t [h=128, b=512] = sum_n (wt chunk t,n).T @ (xt chunk n),
      psum bank t%4, tanh on ACT -> ht_sb[t] (bf16). 128 matmuls at the
      streaming rate (~218 ns each).
  L2: 4-way PE column tiling over h-tiles, N=512: quad q runs tiles
      4q..4q+3 concurrently in column groups 0..3 (tile_position (0,32j)),
      each group accumulating into its own PSUM bank at partitions
      [32j, 32j+10). 8 quads x ~216 ns.
  Tail: DVE add tree (pso0+pso1, pso2+pso3, sum) -> out_sb [10,512] f32,
      single DMA out. Host transposes back.
"""

import os

import numpy as np
import ml_dtypes

BATCH, IN_DIM, HIDDEN, N_MEMBERS, ENS, OUT = 4096, 512, 4096, 256, 64, 10
NCORES = 8
B_LOC = BATCH // NCORES      # 512 batch rows per core
HT = 128                     # h-tile height (PSUM partition dim)
NHT = HIDDEN // HT           # 32 h-tiles
NIC = IN_DIM // 128          # 4 contraction chunks for the first matmul
N_WARM = 38                  # warm-up matmuls (N=128) to lift the HAM gate

# ---- stream column layout (bf16 elems per partition), consumption order --
# [q0 q1 q2 q3 g0] [g1 aw] [g2 g3] [g4..g9] [g10..g15] [g16..g23] [g24..g31]
XT_OFF = [0, 512, 1024, 1536]
AW_OFF = 3072
WT_OFF = {0: 2048, 1: 2560, 2: 3392, 3: 3904}
for _t in range(4, NHT):
    WT_OFF[_t] = 4416 + (_t - 4) * 512
STREAM_COLS = 18752
# (col_start, col_end, ring): A=Sync, B=Scalar, C=GpSimd.
# The Sync ring activates ~1us after its doorbell but gets starved by the
# SDMA arbiter when the other rings have work, so it carries the
# latency-critical head+mid while Scalar/GpSimd (2.5-4us activation lag,
# deliberately late doorbells) carry the two tail blocks.
DMA_SPLITS = [
    (0, 2560, "A"),       # F0: q0..q3 g0   (critical head)
    (2560, 3392, "A"),    # F1: g1 aw
    (3392, 4416, "A"),    # F2: g2 g3
    (4416, 7488, "A"),    # F3: g4..g9
    (7488, 10560, "A"),   # F4: g10..g15
    (10560, 14656, "B"),  # F5: g16..g23
    (14656, 18752, "C"),  # F6: g24..g31
]
# first L1 tile that needs DMA i
TILE_DMA = {0: 0, 1: 1, 2: 2, 4: 3, 10: 4, 16: 5, 24: 6}

_compiled = None
LAST_RESULT = None           # BassKernelResults of the most recent run


def _build():
    """Hand-scheduled raw Bass (no Tile framework — its prologue/epilogue
    cost ~12us on a kernel this size). 12 manual semaphores; the
    walrus-injected epilogue handles queue drain + semaphore-file reset."""
    from concourse import bacc, mybir

    bf16 = mybir.dt.bfloat16
    f32 = mybir.dt.float32

    nc = bacc.Bacc(
        "TRN2",
        target_bir_lowering=False,
        debug=False,
        enable_asserts=False,
        num_devices=NCORES,
    )
    stream = nc.dram_tensor("stream", [128, STREAM_COLS], bf16, kind="ExternalInput")
    out = nc.dram_tensor("out", [OUT, 4 * B_LOC], bf16, kind="ExternalOutput")

    warm_sb = nc.alloc_sbuf_tensor("warm_sb", [128, 128], mybir.dt.uint16)
    stream_sb = nc.alloc_sbuf_tensor("stream_sb", [128, STREAM_COLS], bf16)
    ht_sb = [
        nc.alloc_sbuf_tensor(f"ht_sb{t}", [128, B_LOC], bf16) for t in range(NHT)
    ]
    out_sb = nc.alloc_sbuf_tensor("out_sb", [OUT, 4 * B_LOC], bf16)

    # 8 PSUM banks: 4 for L1 accumulation (t%4), 4 for the L2 column groups
    # (group j accumulates h-tiles k≡j mod 4 at partitions [32j, 32j+10)).
    # Warm-up matmuls borrow ps[3] (first real use is L1 t=3).
    ps = [nc.alloc_psum_tensor(f"ps{k}", [128, B_LOC], f32) for k in range(4)]
    pso = [nc.alloc_psum_tensor(f"pso{j}", [128, B_LOC], f32) for j in range(4)]

    s_d = [nc.alloc_semaphore(f"s_d{i}") for i in range(len(DMA_SPLITS))]
    s_out = nc.alloc_semaphore("s_out")
    sg = nc.alloc_semaphore("sg")    # warm tile ready
    sm = nc.alloc_semaphore("sm")    # L1 tile t accumulated
    sa = nc.alloc_semaphore("sa")    # tanh t done
    sm2 = nc.alloc_semaphore("sm2")  # all L2 matmuls done
    sv = nc.alloc_semaphore("sv")    # L2 accumulator copies done

    tanh = mybir.ActivationFunctionType.Tanh

    # ---- input DMAs
    sview = stream.ap()

    def dma(i):
        c0, c1, ring = DMA_SPLITS[i]
        eng = {"A": nc.sync, "B": nc.scalar, "C": nc.gpsimd}[ring]
        eng.dma_start(
            out=stream_sb.ap()[:, c0:c1], in_=sview[:, c0:c1]
        ).then_inc(s_d[i], 16)

    for i in (0, 1, 2, 3, 4):
        dma(i)                             # Sync ring, consumption order

    # ---- DVE: warm-up fill. The PE HAM clock-gate watches real datapath
    # activity (zeros don't count), so the warm-up needs varying nonzero
    # data: random bits masked to bf16 in [1, 2). Warm-up matmuls measured
    # NOT to slow the concurrent head DMA (v2 vs v4 traces: ~190 GB/s both).
    if os.environ.get("KERNEL_SIMSAFE") == "1":
        fill = nc.vector.memset(warm_sb.ap(), 0x3F80)  # CoreSim xorwow workaround
    else:
        fill = nc.vector.random(warm_sb.ap())
    fill.then_inc(sg, 1)
    nc.vector.wait_ge(sg, 1)
    nc.vector.tensor_scalar(
        out=warm_sb.ap(),
        in0=warm_sb.ap(),
        scalar1=0x007F,
        scalar2=0x3F80,
        op0=mybir.AluOpType.bitwise_and,
        op1=mybir.AluOpType.bitwise_or,
    ).then_inc(sg, 1)

    # ---- PE
    pe = nc.tensor
    pe.wait_ge(sg, 2)
    warm_bf = warm_sb.ap().bitcast(bf16)
    for _ in range(N_WARM):
        pe.matmul(
            out=ps[3].ap()[:, :128], lhsT=warm_bf, rhs=warm_bf,
            start=True, stop=True,
        )

    # L1: 128 back-to-back N=512 matmuls
    for t in range(NHT):
        if t in TILE_DMA:
            pe.wait_ge(s_d[TILE_DMA[t]], 16)
        if t >= 4:
            pe.wait_ge(sa, t - 3)          # psum bank free after tanh(t-4)
        for n in range(NIC):
            mm = pe.matmul(
                out=ps[t % 4].ap(),
                lhsT=stream_sb.ap()[:, WT_OFF[t] + 128 * n : WT_OFF[t] + 128 * (n + 1)],
                rhs=stream_sb.ap()[:, XT_OFF[n] : XT_OFF[n] + B_LOC],
                start=(n == 0),
                stop=(n == NIC - 1),
            )
        mm.then_inc(sm, 1)

    # L2 phase: 8 pipelined quads, 4 column groups x N=512 each
    for q in range(NHT // 4):
        pe.wait_ge(sa, 4 * q + 4)          # tanh of tiles 4q..4q+3 done
        for j in range(4):
            k = 4 * q + j
            mm = pe.matmul(
                out=pso[j].ap()[32 * j : 32 * j + OUT, :],
                lhsT=stream_sb.ap()[:, AW_OFF + OUT * k : AW_OFF + OUT * (k + 1)],
                rhs=ht_sb[k].ap(),
                start=(q == 0),
                stop=(q == NHT // 4 - 1),
                tile_position=(0, 32 * j),
            )
    mm.then_inc(sm2, 1)                    # MMs retire in pc order

    # ---- ACT: issue the Scalar-ring input DMAs, then tanh PSUM->SBUF per
    # h-tile (act table preloaded by the compiler-injected ACT_TABLE_LOAD),
    # then evacuate L2 groups 0/2 while DVE evacuates 1/3 (different banks;
    # the four partial sums are added on the host — copies run at the DVE/
    # ACT fast-copy rate while adds would cost ~2.5x).
    act = nc.scalar
    act.wait_ge(sm, 1)
    dma(5)                                 # F5 doorbell after tile 0
    for t in range(NHT):
        if t > 0:
            act.wait_ge(sm, t + 1)
        act.activation(out=ht_sb[t].ap(), in_=ps[t % 4].ap(), func=tanh).then_inc(
            sa, 1
        )
    copyf = mybir.ActivationFunctionType.Copy
    act.wait_ge(sm2, 1)
    act.activation(
        out=out_sb.ap()[:, 0:B_LOC], in_=pso[0].ap()[0:OUT, :], func=copyf
    )
    act.activation(
        out=out_sb.ap()[:, 2 * B_LOC : 3 * B_LOC],
        in_=pso[2].ap()[64 : 64 + OUT, :],
        func=copyf,
    ).then_inc(sv, 1)

    # ---- GpSimd: late tail-block doorbell (SWDGE), same contention logic
    nc.gpsimd.wait_ge(sm, 4)
    dma(6)

    # ---- DVE: evacuate L2 groups 1/3
    v = nc.vector
    v.wait_ge(sm2, 1)
    v.tensor_copy(
        out=out_sb.ap()[:, B_LOC : 2 * B_LOC],
        in_=pso[1].ap()[32 : 32 + OUT, :],
    )
    v.tensor_copy(
        out=out_sb.ap()[:, 3 * B_LOC :],
        in_=pso[3].ap()[96 : 96 + OUT, :],
    ).then_inc(sv, 1)

    # ---- Sync tail: result out. No completion wait / sem reset — the
    # injected per-engine epilogue drains every queue after program end.
    nc.sync.wait_ge(sv, 2)
    nc.sync.dma_start(out=out.ap(), in_=out_sb.ap()).then_inc(s_out, 16)

    nc.compile()
    return nc


def _pack_host(x, W_in, W_pred, ids):
    """Build the per-core stream tensors from a shared template."""
    A = np.zeros((HIDDEN, OUT), dtype=np.float64)
    np.add.at(
        A,
        ids.reshape(-1),
        W_pred.transpose(0, 2, 1).reshape(-1, OUT).astype(np.float64),
    )
    A /= N_MEMBERS
    a_packed = np.ascontiguousarray(
        A.reshape(NHT, 128, OUT).transpose(1, 0, 2).reshape(128, NHT * OUT)
    ).astype(ml_dtypes.bfloat16)

    wt_bf = W_in.T.astype(ml_dtypes.bfloat16)                  # [512, 4096]
    # wt tile t chunk: [p, n*128+h] = W_in[t*128+h, n*128+p]
    wt_tiles = wt_bf.reshape(NIC, 128, NHT, HT).transpose(1, 2, 0, 3)  # [p,t,n,h]

    template = np.zeros((128, STREAM_COLS), dtype=ml_dtypes.bfloat16)
    for t in range(NHT):
        template[:, WT_OFF[t] : WT_OFF[t] + 512] = wt_tiles[:, t].reshape(128, 512)
    template[:, AW_OFF : AW_OFF + NHT * OUT] = a_packed

    xt_bf = x.T.astype(ml_dtypes.bfloat16)                     # [512, 4096]
    streams = []
    for c in range(NCORES):
        xs = xt_bf[:, c * B_LOC : (c + 1) * B_LOC]             # [512, 512]
        xq = xs.reshape(NIC, 128, B_LOC)                       # [n, p, b]
        s = template.copy()
        for n in range(NIC):
            s[:, XT_OFF[n] : XT_OFF[n] + B_LOC] = xq[n]
        streams.append(s)
    return streams


def kernel(**inputs) -> np.ndarray:
    x = np.asarray(inputs["x"], dtype=np.float32)              # [4096, 512]
    W_in = np.asarray(inputs["W_in"], dtype=np.float32)        # [4096, 512]
    W_pred = np.asarray(inputs["W_pred"], dtype=np.float32)    # [256, 10, 64]
    ids = np.asarray(inputs["ensemble_input_ids"])             # [256, 64] int32

    streams = _pack_host(x, W_in, W_pred, ids)

    global _compiled
    if _compiled is None:
        _compiled = _build()
    nc = _compiled

    in_maps = [{"stream": streams[c]} for c in range(NCORES)]

    from concourse.bass_utils import run_bass_kernel_spmd

    trace = bool(int(os.environ.get("KERNEL_TRACE", "0")))
    res = run_bass_kernel_spmd(
        nc, in_maps, core_ids=list(range(NCORES)), trace=trace
    )
    global LAST_RESULT
    LAST_RESULT = res

    out = np.empty((BATCH, OUT), dtype=np.float32)
    for c in range(NCORES):
        arr = np.asarray(res.results[c]["out"], dtype=np.float32)  # [10, 2048]
        acc = (
            arr[:, 0:B_LOC]
            + arr[:, B_LOC : 2 * B_LOC]
            + arr[:, 2 * B_LOC : 3 * B_LOC]
            + arr[:, 3 * B_LOC :]
        )
        out[c * B_LOC : (c + 1) * B_LOC, :] = acc.T
    return out
